# revision 1
# baseline (speedup 1.0000x reference)
"""Trainium2 Bass kernel for nn_MixChan (dense_mlp).

Reference computation (per batch sample b):
    d   = dist / dist.sum()                       # (32,)
    xs  = x.sum(axis=K) * d[c]                    # (32, 512, 512)
    ds  = avgpool4x4(xs)                          # (32, 128, 128)
    h1  = leaky_relu(ds.flat @ W1.T + b1, 0.2)    # (32, 1024)
    coef= leaky_relu(h1 @ W2.T + b2, 0.2)         # (32, 1)
    out = einsum('c,cwh->wh', coef, xs) / 32      # (512, 512)

Sharding: data-parallel over batch B=8 across the 8 NeuronCores; the MLP
weights are replicated (W1 is streamed from HBM as bf16).

Per-core dataflow (one pass over x, fused):
  for each group g of 4 image rows (128 groups):
    - DMA x[c,k,4g:4g+4,:] into SBUF as [128p=(w4,c32), (k2,h512)] fp32
    - DMA W1.T rows [128f, 1024h] bf16
    - DVE: k-sum -> xs slab slice [128, 512] bf16 (kept in SBUF, 16 MiB)
    - DVE: reduce over h-groups of 4 -> red4 [128, 128] fp32
    - PE : red4.T @ A -> pooledT [128f, 32c] (A folds the w-sum, the /16
           avgpool normalization and the per-channel dist weight d_c)
    - ACT: copy/cast pooledT -> bf16
    - PE : h1 += pooledT.T @ W1T chunk (PSUM accumulate over all 128 chunks)
  MLP tail (tiny), building S [128, 4] = u_c * delta(w, m), u = coef*d/32
  for each group g:
    - PE : out rows = S.T @ xs slab slice  (the weighted channel sum)
    - ACT/DVE: copy PSUM -> SBUF, DMA out
"""

import numpy as np
import ml_dtypes

B, C, K, W = 8, 32, 2, 512
P = 4                 # pool kernel/stride
G = W // P            # 128 row groups
FEAT = G * G          # 16384
HID = 1024
NEG = 0.2
N_CORES = 8

_prog_cache = {}


def _build_program():
    import concourse.bass as bass
    import concourse.tile as tile
    from concourse import bacc, mybir

    dt = mybir.dt
    f32 = dt.float32
    bf16 = dt.bfloat16
    Alu = mybir.AluOpType

    nc = bacc.Bacc(
        "TRN2",
        debug=False,
        enable_asserts=False,
        target_bir_lowering=False,
        num_devices=N_CORES,
    )

    x_t = nc.dram_tensor("x", [C, K, W, W], f32, kind="ExternalInput").ap()
    w1t_t = nc.dram_tensor("w1t", [FEAT, HID], bf16, kind="ExternalInput").ap()
    amat_t = nc.dram_tensor("amat", [128, C], f32, kind="ExternalInput").ap()
    m0_t = nc.dram_tensor("m0", [128, P], bf16, kind="ExternalInput").ap()
    t128_t = nc.dram_tensor("t128", [128, 128], f32, kind="ExternalInput").ap()
    b1r_t = nc.dram_tensor("b1r", [C, HID], f32, kind="ExternalInput").ap()
    w2r_t = nc.dram_tensor("w2r", [C, HID], f32, kind="ExternalInput").ap()
    b2r_t = nc.dram_tensor("b2r", [C, 1], f32, kind="ExternalInput").ap()
    out_t = nc.dram_tensor("out", [W, W], f32, kind="ExternalOutput").ap()

    with tile.TileContext(nc) as tc:
        with (
            tc.tile_pool(name="singles", bufs=1) as singles,
            tc.tile_pool(name="small", bufs=1) as small,
        ):
            # constants
            amat_sb = singles.tile([128, C], f32)
            nc.sync.dma_start(amat_sb[:], amat_t)
            m0_sb = singles.tile([128, P], bf16)
            nc.sync.dma_start(m0_sb[:], m0_t)
            t128_sb = singles.tile([128, 128], f32)
            nc.sync.dma_start(t128_sb[:], t128_t)
            b1r_sb = singles.tile([C, HID], f32)
            nc.sync.dma_start(b1r_sb[:], b1r_t)
            w2r_sb = singles.tile([C, HID], f32)
            nc.sync.dma_start(w2r_sb[:], w2r_t)
            b2r_sb = singles.tile([C, 1], f32)
            nc.sync.dma_start(b2r_sb[:], b2r_t)

            # xs slab: k-summed (unscaled) x, bf16, [(w4 c32), (g128 h512)]
            xs_sb = singles.tile([128, G * W], bf16)
            S_sb = singles.tile([128, P], bf16)

            with tc.tile_pool(name="psum_h1", bufs=1, space="PSUM") as ph1:
                h1p = ph1.tile([C, HID], f32)

                with (
                    tc.tile_pool(name="xt", bufs=3) as xtp,
                    tc.tile_pool(name="w1c", bufs=3) as w1p,
                    tc.tile_pool(name="red", bufs=2) as redp,
                    tc.tile_pool(name="ptsb", bufs=2) as ptp,
                    tc.tile_pool(name="psum_pt", bufs=2, space="PSUM") as pptp,
                ):
                    for g in range(G):
                        xt = xtp.tile([128, K, W], f32)
                        # DRAM side iterates (w, c, k, h) to match the
                        # partition-major order p = w*32 + c.
                        xg = x_t[:, :, P * g : P * g + P, :].transpose([2, 0, 1, 3])
                        nc.sync.dma_start(xt[:], xg)

                        w1c = w1p.tile([128, HID], bf16)
                        nc.sync.dma_start(w1c[:], w1t_t[128 * g : 128 * (g + 1), :])

                        xs_slice = xs_sb[:, W * g : W * (g + 1)]
                        nc.vector.tensor_add(xs_slice, xt[:, 0, :], xt[:, 1, :])

                        red4 = redp.tile([128, G], f32)
                        nc.vector.reduce_sum(
                            red4[:],
                            xs_slice.rearrange("p (j f) -> p j f", f=P),
                            axis=mybir.AxisListType.X,
                        )

                        ptps = pptp.tile([128, C], f32)
                        nc.tensor.matmul(
                            ptps[:], lhsT=red4[:], rhs=amat_sb[:],
                            start=True, stop=True,
                        )
                        ptsb = ptp.tile([128, C], bf16)
                        nc.scalar.copy(ptsb[:], ptps[:])

                        first, last = g == 0, g == G - 1
                        nc.tensor.matmul(
                            h1p[:, 0:512], lhsT=ptsb[:], rhs=w1c[:, 0:512],
                            start=first, stop=last,
                        )
                        nc.tensor.matmul(
                            h1p[:, 512:1024], lhsT=ptsb[:], rhs=w1c[:, 512:1024],
                            start=first, stop=last,
                        )

                # ---- MLP tail ----
                tmp1 = small.tile([C, HID], f32)
                nc.vector.tensor_add(tmp1[:], h1p[:], b1r_sb[:])
                h1s = small.tile([C, HID], f32)
                nc.vector.scalar_tensor_tensor(
                    out=h1s[:], in0=tmp1[:], scalar=NEG, in1=tmp1[:],
                    op0=Alu.mult, op1=Alu.max,
                )
                # coef = leaky(h1s @ W2.T + b2): tensor_tensor_reduce crashes
                # the exec unit on HW, so mul + reduce + add instead.
                prod = small.tile([C, HID], f32)
                nc.vector.tensor_mul(prod[:], h1s[:], w2r_sb[:])
                red = small.tile([C, 1], f32)
                nc.vector.reduce_sum(red[:], prod[:], axis=mybir.AxisListType.X)
                cf = small.tile([C, 1], f32)
                nc.vector.tensor_add(cf[:], red[:], b2r_sb[:])
                # coef padded to 128 partitions for a K=128 matmul (small-K
                # matmuls are flaky on HW).
                coef128 = small.tile([128, 1], f32)
                nc.vector.memset(coef128[:], 0.0)
                nc.vector.scalar_tensor_tensor(
                    out=coef128[0:C, :], in0=cf[:], scalar=NEG, in1=cf[:],
                    op0=Alu.mult, op1=Alu.max,
                )
                with tc.tile_pool(name="psum_u", bufs=1, space="PSUM") as pup:
                    u_ps = pup.tile([128, 1], f32)
                    # u128[p] = coef[p%32] * d[p%32] / 32  (t128 folds d/32)
                    nc.tensor.matmul(
                        u_ps[:], lhsT=t128_sb[:], rhs=coef128[:],
                        start=True, stop=True,
                    )
                    nc.vector.tensor_scalar_mul(S_sb[:], m0_sb[:], u_ps[:])

            # ---- weighted channel sum ----
            with (
                tc.tile_pool(name="psum_o", bufs=6, space="PSUM") as pop,
                tc.tile_pool(name="ob", bufs=6) as obp,
            ):
                for g in range(G):
                    po = pop.tile([P, W], f32)
                    nc.tensor.matmul(
                        po[:], lhsT=S_sb[:], rhs=xs_sb[:, W * g : W * (g + 1)],
                        start=True, stop=True,
                    )
                    ob = obp.tile([P, W], f32)
                    if g % 2 == 0:
                        nc.scalar.copy(ob[:], po[:])
                    else:
                        nc.vector.tensor_copy(ob[:], po[:])
                    nc.sync.dma_start(out_t[P * g : P * (g + 1), :], ob[:])

    nc.compile()
    return nc


def _get_program():
    if "nc" not in _prog_cache:
        _prog_cache["nc"] = _build_program()
    return _prog_cache["nc"]


def prep_in_maps(x, dist, W1, b1, W2, b2):
    bf16 = ml_dtypes.bfloat16
    x = np.asarray(x, dtype=np.float32)
    dist = np.asarray(dist, dtype=np.float32)
    W1 = np.asarray(W1, dtype=np.float32)
    b1 = np.asarray(b1, dtype=np.float32)
    W2 = np.asarray(W2, dtype=np.float32)
    b2 = np.asarray(b2, dtype=np.float32)

    d = dist / dist.sum()
    # A[(w*32+c), n] = d[c]/16 * delta(c, n): folds w-sum + avgpool norm + d
    amat = np.tile(np.diag(d / (P * P)).astype(np.float32), (P, 1))
    # M0[(w*32+c), m] = delta(w, m)
    m0 = np.repeat(np.eye(P, dtype=np.float32), C, axis=0).astype(bf16)
    # T128[c, p] = d[c]/32 * delta(c, p % 32), zero-padded to K=128 rows
    t128 = np.zeros((128, 128), np.float32)
    t128[0:C] = np.tile(np.diag(d / C).astype(np.float32), (1, P))
    b1r = np.ascontiguousarray(np.broadcast_to(b1, (C, HID))).astype(np.float32)
    w2r = np.ascontiguousarray(np.broadcast_to(W2[0], (C, HID))).astype(np.float32)
    b2r = np.full((C, 1), b2[0], dtype=np.float32)
    w1t = np.ascontiguousarray(W1.T).astype(bf16)

    return [
        dict(
            x=np.ascontiguousarray(x[b]),
            w1t=w1t,
            amat=amat,
            m0=m0,
            t128=t128,
            b1r=b1r,
            w2r=w2r,
            b2r=b2r,
        )
        for b in range(N_CORES)
    ]


def kernel(x, dist, W1, b1, W2, b2):
    from concourse.bass_utils import run_bass_kernel_spmd

    in_maps = prep_in_maps(x, dist, W1, b1, W2, b2)
    nc = _get_program()
    res = run_bass_kernel_spmd(nc, in_maps, list(range(N_CORES)))
    out = np.stack([res.results[i]["out"] for i in range(N_CORES)])
    return out[:, None, :, :].astype(np.float32)



# revision 9
# speedup vs baseline: 2.5136x; 2.5136x over previous
"""Trainium2 Bass kernel for nn_MixChan (dense_mlp).

Reference computation (per batch sample b):
    d   = dist / dist.sum()                       # (32,)
    xs  = x.sum(axis=K) * d[c]                    # (32, 512, 512)
    ds  = avgpool4x4(xs)                          # (32, 128, 128)
    h1  = leaky_relu(ds.flat @ W1.T + b1, 0.2)    # (32, 1024)
    coef= leaky_relu(h1 @ W2.T + b2, 0.2)         # (32, 1)
    out = einsum('c,cwh->wh', coef, xs) / 32      # (512, 512)

Sharding: data-parallel over batch B=8 across the 8 NeuronCores; the MLP
weights are replicated (W1 is streamed from HBM as bf16).

Both x and W1 are pre-transposed on the host into partition-major layouts
so every DMA is a [128-partition, big-contiguous-run] transfer: the HWDGE
splits a DMA across SDMA engines by the outer DRAM dim, so a 128-outer AP
uses all 16 engines (the old (w4,c32,k,h) transpose AP used only 4 and the
kernel ran at ~100 GB/s, DMA-bound at 930 us).

Per-core dataflow (one pass over x, fused), TILE=4 row-groups per step:
  for each tile t (32 tiles):
    - DMA x2[:, t] -> xt [128p=(w4,c32), (g4,k2,h512)] fp32 (2 MiB, 16 KiB/desc)
    - DMA w1x[:, t] -> w14 [128p=j, (g4,hid1024)] bf16 (1 MiB, 8 KiB/desc)
    - DVE: k-sum -> xs slab [128, (g,h)] bf16 (kept in SBUF, 16 MiB)
    - DVE: reduce over h-groups of 4 -> red16 [128, 512] fp32
    - PE x4: red16 chunk.T @ A -> pooled [128f, 32c] into one PSUM bank (A
      folds the w-sum, /16 avgpool norm, and the per-channel dist weight)
    - ACT: cast the 4-group pooled bank -> bf16
    - PE x8: h1 += pooled.T @ W1 chunks (PSUM accumulate over all 128 chunks)
  MLP tail (tiny), building S [128, 4] = u_c * delta(w, m), u = coef*d/32
  for each group g: (output phase)
    - PE : out rows = S.T @ xs slab slice  (the weighted channel sum)
    - ACT/DVE alternating: copy PSUM -> staging [4, 16*512]
    - every 16 groups: one batched DMA staging -> out rows (128 KiB)
"""

import numpy as np
import ml_dtypes

B, C, K, W = 8, 32, 2, 512
P = 4                 # pool kernel/stride
G = W // P            # 128 row groups
FEAT = G * G          # 16384
HID = 1024
NEG = 0.2
N_CORES = 8
TILE = 4              # row groups per main-loop tile
NT = G // TILE        # 32 tiles
OUTB = 16             # groups per output staging buffer

_prog_cache = {}


def _build_program():
    import concourse.bass as bass
    import concourse.tile as tile
    from concourse import bacc, mybir

    dt = mybir.dt
    f32 = dt.float32
    bf16 = dt.bfloat16
    Alu = mybir.AluOpType

    nc = bacc.Bacc(
        "TRN2",
        debug=False,
        enable_asserts=False,
        target_bir_lowering=False,
        num_devices=N_CORES,
    )

    # x2[(w*32+c), ((g*2+k)*512+h)] = x[c,k,4g+w,h]   (host pre-transposed)
    x2_t = nc.dram_tensor("x2", [128, G * K * W], f32, kind="ExternalInput").ap()
    # w1x[j, (g*1024+hid)] = W1[hid, 128g+j]          (host pre-transposed)
    w1x_t = nc.dram_tensor("w1x", [128, G * HID], bf16, kind="ExternalInput").ap()
    amat_t = nc.dram_tensor("amat", [128, C], f32, kind="ExternalInput").ap()
    m0_t = nc.dram_tensor("m0", [128, P], bf16, kind="ExternalInput").ap()
    t128_t = nc.dram_tensor("t128", [128, 128], f32, kind="ExternalInput").ap()
    b1r_t = nc.dram_tensor("b1r", [C, HID], f32, kind="ExternalInput").ap()
    w2r_t = nc.dram_tensor("w2r", [C, HID], f32, kind="ExternalInput").ap()
    b2r_t = nc.dram_tensor("b2r", [C, 1], f32, kind="ExternalInput").ap()
    out_t = nc.dram_tensor("out", [W, W], f32, kind="ExternalOutput").ap()

    with tile.TileContext(nc) as tc:
        with tc.tile_pool(name="singles", bufs=1) as singles:
            # constants
            amat_sb = singles.tile([128, C], f32)
            nc.sync.dma_start(amat_sb[:], amat_t)
            m0_sb = singles.tile([128, P], bf16)
            nc.sync.dma_start(m0_sb[:], m0_t)
            t128_sb = singles.tile([128, 128], f32)
            nc.sync.dma_start(t128_sb[:], t128_t)
            b1r_sb = singles.tile([C, HID], f32)
            nc.sync.dma_start(b1r_sb[:], b1r_t)
            w2r_sb = singles.tile([C, HID], f32)
            nc.sync.dma_start(w2r_sb[:], w2r_t)
            b2r_sb = singles.tile([C, 1], f32)
            nc.sync.dma_start(b2r_sb[:], b2r_t)

            # xs slab: k-summed (unscaled) x, bf16, [(w4 c32), (g128 h512)]
            xs_sb = singles.tile([128, G * W], bf16)
            S_sb = singles.tile([128, P], bf16)

            with tc.tile_pool(name="psum_h1", bufs=1, space="PSUM") as ph1:
                h1p = ph1.tile([C, HID], f32)

                with (
                    tc.tile_pool(name="xt", bufs=2) as xtp,
                    tc.tile_pool(name="w1c", bufs=2) as w1p,
                    tc.tile_pool(name="red", bufs=2) as redp,
                    tc.tile_pool(name="ptsb", bufs=2) as ptp,
                    tc.tile_pool(name="psum_pt", bufs=2, space="PSUM") as pptp,
                ):
                    for t in range(NT):
                        xt = xtp.tile([128, TILE, K, W], f32)
                        nc.sync.dma_start(
                            xt[:],
                            x2_t[:, TILE * K * W * t : TILE * K * W * (t + 1)],
                        )
                        w14 = w1p.tile([128, TILE, HID], bf16)
                        nc.scalar.dma_start(
                            w14[:],
                            w1x_t[:, TILE * HID * t : TILE * HID * (t + 1)],
                        )

                        # k-sum -> xs (bf16), one instr for the whole tile
                        xs_slice = xs_sb[
                            :, TILE * W * t : TILE * W * (t + 1)
                        ].rearrange("p (g h) -> p g h", h=W)
                        nc.vector.tensor_add(
                            xs_slice, xt[:, :, 0, :], xt[:, :, 1, :]
                        )

                        # 4:1 reduce over h within pooling groups
                        red16 = redp.tile([128, TILE * G], f32)
                        nc.vector.reduce_sum(
                            red16[:],
                            xs_sb[
                                :, TILE * W * t : TILE * W * (t + 1)
                            ].rearrange("p (j f) -> p j f", f=P),
                            axis=mybir.AxisListType.X,
                        )

                        # pooled features for the 4 groups -> one PSUM bank
                        pp4 = pptp.tile([128, TILE, C], f32)
                        for q in range(TILE):
                            nc.tensor.matmul(
                                pp4[:, q, :],
                                lhsT=red16[:, G * q : G * (q + 1)],
                                rhs=amat_sb[:],
                                start=True,
                                stop=True,
                            )
                        pt4 = ptp.tile([128, TILE, C], bf16)
                        nc.scalar.copy(pt4[:], pp4[:])

                        for q in range(TILE):
                            first = t == 0 and q == 0
                            last = t == NT - 1 and q == TILE - 1
                            nc.tensor.matmul(
                                h1p[:, 0:512],
                                lhsT=pt4[:, q, :],
                                rhs=w14[:, q, 0:512],
                                start=first,
                                stop=last,
                            )
                            nc.tensor.matmul(
                                h1p[:, 512:1024],
                                lhsT=pt4[:, q, :],
                                rhs=w14[:, q, 512:1024],
                                start=first,
                                stop=last,
                            )

                # ---- MLP tail ----
                small_cm = tc.tile_pool(name="small", bufs=1)
                small = small_cm.__enter__()
                tmp1 = small.tile([C, HID], f32)
                nc.vector.tensor_add(tmp1[:], h1p[:], b1r_sb[:])
                h1s = small.tile([C, HID], f32)
                nc.vector.scalar_tensor_tensor(
                    out=h1s[:], in0=tmp1[:], scalar=NEG, in1=tmp1[:],
                    op0=Alu.mult, op1=Alu.max,
                )
                # coef = leaky(h1s @ W2.T + b2): tensor_tensor_reduce crashes
                # the exec unit on HW, so mul + reduce + add instead.
                prod = small.tile([C, HID], f32)
                nc.vector.tensor_mul(prod[:], h1s[:], w2r_sb[:])
                red = small.tile([C, 1], f32)
                nc.vector.reduce_sum(red[:], prod[:], axis=mybir.AxisListType.X)
                cf = small.tile([C, 1], f32)
                nc.vector.tensor_add(cf[:], red[:], b2r_sb[:])
                # coef padded to 128 partitions for a K=128 matmul (small-K
                # matmuls are flaky on HW).
                coef128 = small.tile([128, 1], f32)
                nc.vector.memset(coef128[:], 0.0)
                nc.vector.scalar_tensor_tensor(
                    out=coef128[0:C, :], in0=cf[:], scalar=NEG, in1=cf[:],
                    op0=Alu.mult, op1=Alu.max,
                )
                with tc.tile_pool(name="psum_u", bufs=1, space="PSUM") as pup:
                    u_ps = pup.tile([128, 1], f32)
                    # u128[p] = coef[p%32] * d[p%32] / 32  (t128 folds d/32)
                    nc.tensor.matmul(
                        u_ps[:], lhsT=t128_sb[:], rhs=coef128[:],
                        start=True, stop=True,
                    )
                    nc.vector.tensor_scalar_mul(S_sb[:], m0_sb[:], u_ps[:])
                small_cm.__exit__(None, None, None)

            # ---- weighted channel sum ----
            with (
                tc.tile_pool(name="psum_o", bufs=6, space="PSUM") as pop,
                tc.tile_pool(name="stg", bufs=2) as stgp,
            ):
                for blk in range(G // OUTB):
                    stg = stgp.tile([P, OUTB, W], f32)
                    for i in range(OUTB):
                        g = OUTB * blk + i
                        po = pop.tile([P, W], f32)
                        nc.tensor.matmul(
                            po[:], lhsT=S_sb[:],
                            rhs=xs_sb[:, W * g : W * (g + 1)],
                            start=True, stop=True,
                        )
                        if g % 2 == 0:
                            nc.scalar.copy(stg[:, i, :], po[:])
                        else:
                            nc.vector.tensor_copy(stg[:, i, :], po[:])
                    # one batched DMA per OUTB groups: DRAM rows
                    # [4*OUTB*blk, 4*OUTB*(blk+1)) viewed as (w, g, h)
                    rows = out_t[
                        P * OUTB * blk : P * OUTB * (blk + 1), :
                    ].rearrange("(g w) h -> w g h", w=P)
                    nc.sync.dma_start(rows, stg[:])

    nc.compile()
    return nc


def _get_program():
    if "nc" not in _prog_cache:
        _prog_cache["nc"] = _build_program()
    return _prog_cache["nc"]


def prep_in_maps(x, dist, W1, b1, W2, b2):
    bf16 = ml_dtypes.bfloat16
    x = np.asarray(x, dtype=np.float32)
    dist = np.asarray(dist, dtype=np.float32)
    W1 = np.asarray(W1, dtype=np.float32)
    b1 = np.asarray(b1, dtype=np.float32)
    W2 = np.asarray(W2, dtype=np.float32)
    b2 = np.asarray(b2, dtype=np.float32)

    d = dist / dist.sum()
    # A[(w*32+c), n] = d[c]/16 * delta(c, n): folds w-sum + avgpool norm + d
    amat = np.tile(np.diag(d / (P * P)).astype(np.float32), (P, 1))
    # M0[(w*32+c), m] = delta(w, m)
    m0 = np.repeat(np.eye(P, dtype=np.float32), C, axis=0).astype(bf16)
    # T128[c, p] = d[c]/32 * delta(c, p % 32), zero-padded to K=128 rows
    t128 = np.zeros((128, 128), np.float32)
    t128[0:C] = np.tile(np.diag(d / C).astype(np.float32), (1, P))
    b1r = np.ascontiguousarray(np.broadcast_to(b1, (C, HID))).astype(np.float32)
    w2r = np.ascontiguousarray(np.broadcast_to(W2[0], (C, HID))).astype(np.float32)
    b2r = np.full((C, 1), b2[0], dtype=np.float32)

    # w1x[j, g, hid] = W1[hid, 128g+j]  -> [128, G*HID] bf16, so each
    # partition j reads contiguous runs (one 2 KiB run per group)
    w1x = np.ascontiguousarray(
        W1.reshape(HID, G, 128).transpose(2, 1, 0)
    ).astype(bf16).reshape(128, G * HID)

    # x2[(w,c), (g,k,h)] = x[c,k,4g+w,h] -> [128, G*K*W] fp32: partition-
    # major contiguous so DMAs use 16 KiB descriptors on all 16 engines
    x2 = [
        np.ascontiguousarray(
            x[b].reshape(C, K, G, P, W).transpose(3, 0, 2, 1, 4)
        ).reshape(128, G * K * W)
        for b in range(N_CORES)
    ]

    return [
        dict(
            x2=x2[b],
            w1x=w1x,
            amat=amat,
            m0=m0,
            t128=t128,
            b1r=b1r,
            w2r=w2r,
            b2r=b2r,
        )
        for b in range(N_CORES)
    ]


def kernel(x, dist, W1, b1, W2, b2):
    from concourse.bass_utils import run_bass_kernel_spmd

    in_maps = prep_in_maps(x, dist, W1, b1, W2, b2)
    nc = _get_program()
    res = run_bass_kernel_spmd(nc, in_maps, list(range(N_CORES)))
    out = np.stack([res.results[i]["out"] for i in range(N_CORES)])
    return out[:, None, :, :].astype(np.float32)


# revision 12
# speedup vs baseline: 2.8412x; 1.1303x over previous
"""Trainium2 Bass kernel for nn_MixChan (dense_mlp).

Reference computation (per batch sample b):
    d   = dist / dist.sum()                       # (32,)
    xs  = x.sum(axis=K) * d[c]                    # (32, 512, 512)
    ds  = avgpool4x4(xs)                          # (32, 128, 128)
    h1  = leaky_relu(ds.flat @ W1.T + b1, 0.2)    # (32, 1024)
    coef= leaky_relu(h1 @ W2.T + b2, 0.2)         # (32, 1)
    out = einsum('c,cwh->wh', coef, xs) / 32      # (512, 512)

Sharding: data-parallel over batch B=8 across the 8 NeuronCores; the MLP
weights are replicated (W1 is streamed from HBM as bf16).

Both x and W1 are pre-transposed on the host into partition-major layouts
so every DMA is a [128-partition, big-contiguous-run] transfer: the HWDGE
splits a DMA across SDMA engines by the outer DRAM dim, so a 128-outer AP
uses all 16 engines (the old (w4,c32,k,h) transpose AP used only 4 and the
kernel ran at ~100 GB/s, DMA-bound at 930 us).

Per-core dataflow (one pass over x, fused), TILE=4 row-groups per step:
  for each tile t (32 tiles):
    - DMA x2[:, t] -> xt [128p=(w4,c32), (g4,k2,h512)] fp32 (2 MiB, 16 KiB/desc)
    - DMA w1x[:, t] -> w14 [128p=j, (g4,hid1024)] bf16 (1 MiB, 8 KiB/desc)
    - DVE: k-sum -> xs slab [128, (g,h)] bf16 (kept in SBUF, 16 MiB)
    - DVE: reduce over h-groups of 4 -> red16 [128, 512] fp32
    - PE x4: red16 chunk.T @ A -> pooled [128f, 32c] into one PSUM bank (A
      folds the w-sum, /16 avgpool norm, and the per-channel dist weight)
    - ACT: cast the 4-group pooled bank -> bf16
    - PE x8: h1 += pooled.T @ W1 chunks (PSUM accumulate over all 128 chunks)
  MLP tail (tiny), building S [128, 4] = u_c * delta(w, m), u = coef*d/32
  for each group g: (output phase)
    - PE : out rows = S.T @ xs slab slice  (the weighted channel sum)
    - ACT/DVE alternating: copy PSUM -> staging [4, 16*512]
    - every 16 groups: one batched DMA staging -> out rows (128 KiB)
"""

import numpy as np
import ml_dtypes

B, C, K, W = 8, 32, 2, 512
P = 4                 # pool kernel/stride
G = W // P            # 128 row groups
FEAT = G * G          # 16384
HID = 1024
NEG = 0.2
N_CORES = 8
TILE = 4              # row groups per main-loop tile
NT = G // TILE        # 32 tiles
OUTB = 16             # groups per output staging buffer

_prog_cache = {}


def _build_program():
    import concourse.bass as bass
    import concourse.tile as tile
    from concourse import bacc, mybir

    dt = mybir.dt
    f32 = dt.float32
    bf16 = dt.bfloat16
    Alu = mybir.AluOpType

    nc = bacc.Bacc(
        "TRN2",
        debug=False,
        enable_asserts=False,
        target_bir_lowering=False,
        num_devices=N_CORES,
    )

    # x2[(w*32+c), ((g*2+k)*512+h)] = x[c,k,4g+w,h]   (host pre-transposed)
    x2_t = nc.dram_tensor("x2", [128, G * K * W], f32, kind="ExternalInput").ap()
    # w1x[j, (g*1024+hid)] = W1[hid, 128g+j]          (host pre-transposed)
    w1x_t = nc.dram_tensor("w1x", [128, G * HID], bf16, kind="ExternalInput").ap()
    amat_t = nc.dram_tensor("amat", [128, C], f32, kind="ExternalInput").ap()
    m0_t = nc.dram_tensor("m0", [128, P], bf16, kind="ExternalInput").ap()
    t128_t = nc.dram_tensor("t128", [128, 128], f32, kind="ExternalInput").ap()
    b1r_t = nc.dram_tensor("b1r", [C, HID], f32, kind="ExternalInput").ap()
    w2r_t = nc.dram_tensor("w2r", [C, HID], f32, kind="ExternalInput").ap()
    b2r_t = nc.dram_tensor("b2r", [C, 1], f32, kind="ExternalInput").ap()
    out_t = nc.dram_tensor("out", [W, W], f32, kind="ExternalOutput").ap()

    with tile.TileContext(nc) as tc:
        with tc.tile_pool(name="singles", bufs=1) as singles:
            # constants
            amat_sb = singles.tile([128, C], f32)
            nc.sync.dma_start(amat_sb[:], amat_t)
            m0_sb = singles.tile([128, P], bf16)
            nc.sync.dma_start(m0_sb[:], m0_t)
            t128_sb = singles.tile([128, 128], f32)
            nc.sync.dma_start(t128_sb[:], t128_t)
            b1r_sb = singles.tile([C, HID], f32)
            nc.sync.dma_start(b1r_sb[:], b1r_t)
            w2r_sb = singles.tile([C, HID], f32)
            nc.sync.dma_start(w2r_sb[:], w2r_t)
            b2r_sb = singles.tile([C, 1], f32)
            nc.sync.dma_start(b2r_sb[:], b2r_t)

            # xs slab: k-summed (unscaled) x, bf16, [(w4 c32), (g128 h512)]
            xs_sb = singles.tile([128, G * W], bf16)
            S_sb = singles.tile([128, P], bf16)

            with tc.tile_pool(name="psum_h1", bufs=1, space="PSUM") as ph1:
                h1p = ph1.tile([C, HID], f32)

                with (
                    tc.tile_pool(name="xt", bufs=2) as xtp,
                    tc.tile_pool(name="w1c", bufs=3) as w1p,
                    tc.tile_pool(name="red", bufs=2) as redp,
                    tc.tile_pool(name="ptsb", bufs=2) as ptp,
                    tc.tile_pool(name="psum_pt", bufs=2, space="PSUM") as pptp,
                ):
                    for t in range(NT):
                        xt = xtp.tile([128, TILE, K, W], f32)
                        nc.sync.dma_start(
                            xt[:],
                            x2_t[:, TILE * K * W * t : TILE * K * W * (t + 1)],
                        )
                        w14 = w1p.tile([128, TILE, HID], bf16)
                        nc.scalar.dma_start(
                            w14[:],
                            w1x_t[:, TILE * HID * t : TILE * HID * (t + 1)],
                        )

                        # k-sum -> xs (bf16), one instr for the whole tile
                        xs_slice = xs_sb[
                            :, TILE * W * t : TILE * W * (t + 1)
                        ].rearrange("p (g h) -> p g h", h=W)
                        nc.vector.tensor_add(
                            xs_slice, xt[:, :, 0, :], xt[:, :, 1, :]
                        )

                        # 4:1 reduce over h within pooling groups
                        red16 = redp.tile([128, TILE * G], f32)
                        nc.vector.reduce_sum(
                            red16[:],
                            xs_sb[
                                :, TILE * W * t : TILE * W * (t + 1)
                            ].rearrange("p (j f) -> p j f", f=P),
                            axis=mybir.AxisListType.X,
                        )

                        # pooled features for the 4 groups -> one PSUM bank
                        pp4 = pptp.tile([128, TILE, C], f32)
                        for q in range(TILE):
                            nc.tensor.matmul(
                                pp4[:, q, :],
                                lhsT=red16[:, G * q : G * (q + 1)],
                                rhs=amat_sb[:],
                                start=True,
                                stop=True,
                            )
                        pt4 = ptp.tile([128, TILE, C], bf16)
                        nc.scalar.copy(pt4[:], pp4[:])

                        for q in range(TILE):
                            first = t == 0 and q == 0
                            last = t == NT - 1 and q == TILE - 1
                            nc.tensor.matmul(
                                h1p[:, 0:512],
                                lhsT=pt4[:, q, :],
                                rhs=w14[:, q, 0:512],
                                start=first,
                                stop=last,
                            )
                            nc.tensor.matmul(
                                h1p[:, 512:1024],
                                lhsT=pt4[:, q, :],
                                rhs=w14[:, q, 512:1024],
                                start=first,
                                stop=last,
                            )

                # ---- MLP tail ----
                small_cm = tc.tile_pool(name="small", bufs=1)
                small = small_cm.__enter__()
                tmp1 = small.tile([C, HID], f32)
                nc.vector.tensor_add(tmp1[:], h1p[:], b1r_sb[:])
                h1s = small.tile([C, HID], f32)
                nc.vector.scalar_tensor_tensor(
                    out=h1s[:], in0=tmp1[:], scalar=NEG, in1=tmp1[:],
                    op0=Alu.mult, op1=Alu.max,
                )
                # coef = leaky(h1s @ W2.T + b2): tensor_tensor_reduce crashes
                # the exec unit on HW, so mul + reduce + add instead.
                prod = small.tile([C, HID], f32)
                nc.vector.tensor_mul(prod[:], h1s[:], w2r_sb[:])
                red = small.tile([C, 1], f32)
                nc.vector.reduce_sum(red[:], prod[:], axis=mybir.AxisListType.X)
                cf = small.tile([C, 1], f32)
                nc.vector.tensor_add(cf[:], red[:], b2r_sb[:])
                # coef padded to 128 partitions for a K=128 matmul (small-K
                # matmuls are flaky on HW).
                coef128 = small.tile([128, 1], f32)
                nc.vector.memset(coef128[:], 0.0)
                nc.vector.scalar_tensor_tensor(
                    out=coef128[0:C, :], in0=cf[:], scalar=NEG, in1=cf[:],
                    op0=Alu.mult, op1=Alu.max,
                )
                with tc.tile_pool(name="psum_u", bufs=1, space="PSUM") as pup:
                    u_ps = pup.tile([128, 1], f32)
                    # u128[p] = coef[p%32] * d[p%32] / 32  (t128 folds d/32)
                    nc.tensor.matmul(
                        u_ps[:], lhsT=t128_sb[:], rhs=coef128[:],
                        start=True, stop=True,
                    )
                    nc.vector.tensor_scalar_mul(S_sb[:], m0_sb[:], u_ps[:])
                small_cm.__exit__(None, None, None)

            # ---- weighted channel sum ----
            # Four groups per PSUM bank via col-tiling: group (t, j) of block
            # blk lands at psum partitions [32j, 32j+4) (tile_position is
            # inferred from the out AP's base partition).  One full-bank
            # [128, 512] copy then moves 4 groups at once (vs 4-partition
            # copies), and the bf16 staging halves the SBUF-port-0/1-bound
            # read bytes of the output DMA (gpsimd cast-DMA widens to fp32).
            with (
                tc.tile_pool(name="psum_o", bufs=3, space="PSUM") as pop,
                tc.tile_pool(name="stg", bufs=2) as stgp,
            ):
                for blk in range(G // OUTB):
                    stg = stgp.tile([128, OUTB // P, W], bf16)
                    for t in range(OUTB // P):
                        po = pop.tile([128, W], f32)
                        for j in range(P):
                            g = OUTB * blk + P * t + j
                            nc.tensor.matmul(
                                po[32 * j : 32 * j + P, :], lhsT=S_sb[:],
                                rhs=xs_sb[:, W * g : W * (g + 1)],
                                start=True, stop=True,
                                tile_position=(0, 32 * j),
                            )
                        if t % 2 == 0:
                            nc.scalar.copy(stg[:, t, :], po[:])
                        else:
                            nc.vector.tensor_copy(stg[:, t, :], po[:])
                    # out row 4g+w = 64*blk + 16t + 4j + w: one DMA per j
                    # (partitions [32j, 32j+4)), DRAM side viewed as (w,t,h)
                    blk_rows = out_t[
                        P * OUTB * blk : P * OUTB * (blk + 1), :
                    ].rearrange("(t j w) h -> j w t h", j=P, w=P)
                    for j in range(P):
                        nc.gpsimd.dma_start(
                            blk_rows[j], stg[32 * j : 32 * j + P, :, :]
                        )

    nc.compile()
    return nc


def _get_program():
    if "nc" not in _prog_cache:
        _prog_cache["nc"] = _build_program()
    return _prog_cache["nc"]


def prep_in_maps(x, dist, W1, b1, W2, b2):
    bf16 = ml_dtypes.bfloat16
    x = np.asarray(x, dtype=np.float32)
    dist = np.asarray(dist, dtype=np.float32)
    W1 = np.asarray(W1, dtype=np.float32)
    b1 = np.asarray(b1, dtype=np.float32)
    W2 = np.asarray(W2, dtype=np.float32)
    b2 = np.asarray(b2, dtype=np.float32)

    d = dist / dist.sum()
    # A[(w*32+c), n] = d[c]/16 * delta(c, n): folds w-sum + avgpool norm + d
    amat = np.tile(np.diag(d / (P * P)).astype(np.float32), (P, 1))
    # M0[(w*32+c), m] = delta(w, m)
    m0 = np.repeat(np.eye(P, dtype=np.float32), C, axis=0).astype(bf16)
    # T128[c, p] = d[c]/32 * delta(c, p % 32), zero-padded to K=128 rows
    t128 = np.zeros((128, 128), np.float32)
    t128[0:C] = np.tile(np.diag(d / C).astype(np.float32), (1, P))
    b1r = np.ascontiguousarray(np.broadcast_to(b1, (C, HID))).astype(np.float32)
    w2r = np.ascontiguousarray(np.broadcast_to(W2[0], (C, HID))).astype(np.float32)
    b2r = np.full((C, 1), b2[0], dtype=np.float32)

    # w1x[j, g, hid] = W1[hid, 128g+j]  -> [128, G*HID] bf16, so each
    # partition j reads contiguous runs (one 2 KiB run per group)
    w1x = np.ascontiguousarray(
        W1.reshape(HID, G, 128).transpose(2, 1, 0)
    ).astype(bf16).reshape(128, G * HID)

    # x2[(w,c), (g,k,h)] = x[c,k,4g+w,h] -> [128, G*K*W] fp32: partition-
    # major contiguous so DMAs use 16 KiB descriptors on all 16 engines
    x2 = [
        np.ascontiguousarray(
            x[b].reshape(C, K, G, P, W).transpose(3, 0, 2, 1, 4)
        ).reshape(128, G * K * W)
        for b in range(N_CORES)
    ]

    return [
        dict(
            x2=x2[b],
            w1x=w1x,
            amat=amat,
            m0=m0,
            t128=t128,
            b1r=b1r,
            w2r=w2r,
            b2r=b2r,
        )
        for b in range(N_CORES)
    ]


def kernel(x, dist, W1, b1, W2, b2):
    from concourse.bass_utils import run_bass_kernel_spmd

    in_maps = prep_in_maps(x, dist, W1, b1, W2, b2)
    nc = _get_program()
    res = run_bass_kernel_spmd(nc, in_maps, list(range(N_CORES)))
    out = np.stack([res.results[i]["out"] for i in range(N_CORES)])
    return out[:, None, :, :].astype(np.float32)


# revision 16
# speedup vs baseline: 3.7282x; 1.3122x over previous
"""Trainium2 Bass kernel for nn_MixChan (dense_mlp).

Reference computation (per batch sample b):
    d   = dist / dist.sum()                       # (32,)
    xs  = x.sum(axis=K) * d[c]                    # (32, 512, 512)
    ds  = avgpool4x4(xs)                          # (32, 128, 128)
    h1  = leaky_relu(ds.flat @ W1.T + b1, 0.2)    # (32, 1024)
    coef= leaky_relu(h1 @ W2.T + b2, 0.2)         # (32, 1)
    out = einsum('c,cwh->wh', coef, xs) / 32      # (512, 512)

Sharding: data-parallel over batch B=8 across the 8 NeuronCores; the MLP
weights are replicated (W1 is streamed from HBM as bf16).

Both x and W1 are pre-transposed on the host into partition-major layouts
so every DMA is a [128-partition, big-contiguous-run] transfer: the HWDGE
splits a DMA across SDMA engines by the outer DRAM dim, so a 128-outer AP
uses all 16 engines (the old (w4,c32,k,h) transpose AP used only 4 and the
kernel ran at ~100 GB/s, DMA-bound at 930 us).

Per-core dataflow (one pass over x, fused), TILE=4 row-groups per step:
  for each tile t (32 tiles):
    - DMA x2[:, t] -> xt [128p=(w4,c32), (g4,k2,h512)] fp32 (2 MiB, 16 KiB/desc)
    - DMA w1x[:, t] -> w14 [128p=j, (g4,hid1024)] bf16 (1 MiB, 8 KiB/desc)
    - DVE: k-sum -> xs slab [128, (g,h)] bf16 (kept in SBUF, 16 MiB)
    - DVE: reduce over h-groups of 4 -> red16 [128, 512] fp32
    - PE x4: red16 chunk.T @ A -> pooled [128f, 32c] into one PSUM bank (A
      folds the w-sum, /16 avgpool norm, and the per-channel dist weight)
    - ACT: cast the 4-group pooled bank -> bf16
    - PE x8: h1 += pooled.T @ W1 chunks (PSUM accumulate over all 128 chunks)
  MLP tail (tiny), building S [128, 4] = u_c * delta(w, m), u = coef*d/32
  for each group g: (output phase)
    - PE : out rows = S.T @ xs slab slice  (the weighted channel sum)
    - ACT/DVE alternating: copy PSUM -> staging [4, 16*512]
    - every 16 groups: one batched DMA staging -> out rows (128 KiB)
"""

import numpy as np
import ml_dtypes

B, C, K, W = 8, 32, 2, 512
P = 4                 # pool kernel/stride
G = W // P            # 128 row groups
FEAT = G * G          # 16384
HID = 1024
NEG = 0.2
N_CORES = 8
TILE = 4              # row groups per main-loop tile
NT = G // TILE        # 32 tiles
OUTB = 16             # groups per output staging buffer

_prog_cache = {}


def _build_program():
    import concourse.bass as bass
    import concourse.tile as tile
    from concourse import bacc, mybir

    dt = mybir.dt
    f32 = dt.float32
    bf16 = dt.bfloat16
    Alu = mybir.AluOpType

    nc = bacc.Bacc(
        "TRN2",
        debug=False,
        enable_asserts=False,
        target_bir_lowering=False,
        num_devices=N_CORES,
    )

    # x2[(w*32+c), ((g*2+k)*512+h)] = x[c,k,4g+w,h]   (host pre-transposed,
    # cast to bf16 host-side — same traffic-halving trick as W1)
    x2_t = nc.dram_tensor("x2", [128, G * K * W], bf16, kind="ExternalInput").ap()
    # w1x[j, (g*1024+hid)] = W1[hid, 128g+j]          (host pre-transposed)
    w1x_t = nc.dram_tensor("w1x", [128, G * HID], bf16, kind="ExternalInput").ap()
    amat_t = nc.dram_tensor("amat", [128, C], f32, kind="ExternalInput").ap()
    m0_t = nc.dram_tensor("m0", [128, P], bf16, kind="ExternalInput").ap()
    t128_t = nc.dram_tensor("t128", [128, 128], f32, kind="ExternalInput").ap()
    b1r_t = nc.dram_tensor("b1r", [C, HID], f32, kind="ExternalInput").ap()
    w2r_t = nc.dram_tensor("w2r", [C, HID], f32, kind="ExternalInput").ap()
    b2r_t = nc.dram_tensor("b2r", [C, 1], f32, kind="ExternalInput").ap()
    out_t = nc.dram_tensor("out", [W, W], f32, kind="ExternalOutput").ap()

    with tile.TileContext(nc) as tc:
        with tc.tile_pool(name="singles", bufs=1) as singles:
            # constants
            amat_sb = singles.tile([128, C], f32)
            nc.sync.dma_start(amat_sb[:], amat_t)
            m0_sb = singles.tile([128, P], bf16)
            nc.sync.dma_start(m0_sb[:], m0_t)
            t128_sb = singles.tile([128, 128], f32)
            nc.sync.dma_start(t128_sb[:], t128_t)
            b1r_sb = singles.tile([C, HID], f32)
            nc.sync.dma_start(b1r_sb[:], b1r_t)
            w2r_sb = singles.tile([C, HID], f32)
            nc.sync.dma_start(w2r_sb[:], w2r_t)
            b2r_sb = singles.tile([C, 1], f32)
            nc.sync.dma_start(b2r_sb[:], b2r_t)

            # xs slab: k-summed (unscaled) x, bf16, [(w4 c32), (g128 h512)]
            xs_sb = singles.tile([128, G * W], bf16)
            S_sb = singles.tile([128, P], bf16)

            with tc.tile_pool(name="psum_h1", bufs=1, space="PSUM") as ph1:
                h1p = ph1.tile([C, HID], f32)

                with (
                    tc.tile_pool(name="xt", bufs=3) as xtp,
                    tc.tile_pool(name="w1c", bufs=3) as w1p,
                    tc.tile_pool(name="red", bufs=2) as redp,
                    tc.tile_pool(name="ptsb", bufs=2) as ptp,
                    tc.tile_pool(name="psum_pt", bufs=2, space="PSUM") as pptp,
                ):
                    for t in range(NT):
                        xt = xtp.tile([128, TILE, K, W], bf16)
                        nc.sync.dma_start(
                            xt[:],
                            x2_t[:, TILE * K * W * t : TILE * K * W * (t + 1)],
                        )
                        w14 = w1p.tile([128, TILE, HID], bf16)
                        nc.scalar.dma_start(
                            w14[:],
                            w1x_t[:, TILE * HID * t : TILE * HID * (t + 1)],
                        )

                        # k-sum -> xs (bf16), one instr for the whole tile
                        xs_slice = xs_sb[
                            :, TILE * W * t : TILE * W * (t + 1)
                        ].rearrange("p (g h) -> p g h", h=W)
                        nc.vector.tensor_add(
                            xs_slice, xt[:, :, 0, :], xt[:, :, 1, :]
                        )

                        # 4:1 reduce over h within pooling groups
                        red16 = redp.tile([128, TILE * G], f32)
                        nc.vector.reduce_sum(
                            red16[:],
                            xs_sb[
                                :, TILE * W * t : TILE * W * (t + 1)
                            ].rearrange("p (j f) -> p j f", f=P),
                            axis=mybir.AxisListType.X,
                        )

                        # pooled features for the 4 groups -> one PSUM bank
                        pp4 = pptp.tile([128, TILE, C], f32)
                        for q in range(TILE):
                            nc.tensor.matmul(
                                pp4[:, q, :],
                                lhsT=red16[:, G * q : G * (q + 1)],
                                rhs=amat_sb[:],
                                start=True,
                                stop=True,
                            )
                        pt4 = ptp.tile([128, TILE, C], bf16)
                        nc.scalar.copy(pt4[:], pp4[:])

                        for q in range(TILE):
                            first = t == 0 and q == 0
                            last = t == NT - 1 and q == TILE - 1
                            nc.tensor.matmul(
                                h1p[:, 0:512],
                                lhsT=pt4[:, q, :],
                                rhs=w14[:, q, 0:512],
                                start=first,
                                stop=last,
                            )
                            nc.tensor.matmul(
                                h1p[:, 512:1024],
                                lhsT=pt4[:, q, :],
                                rhs=w14[:, q, 512:1024],
                                start=first,
                                stop=last,
                            )

                # ---- MLP tail ----
                small_cm = tc.tile_pool(name="small", bufs=1)
                small = small_cm.__enter__()
                tmp1 = small.tile([C, HID], f32)
                nc.vector.tensor_add(tmp1[:], h1p[:], b1r_sb[:])
                h1s = small.tile([C, HID], f32)
                nc.vector.scalar_tensor_tensor(
                    out=h1s[:], in0=tmp1[:], scalar=NEG, in1=tmp1[:],
                    op0=Alu.mult, op1=Alu.max,
                )
                # coef = leaky(h1s @ W2.T + b2): tensor_tensor_reduce crashes
                # the exec unit on HW, so mul + reduce + add instead.
                prod = small.tile([C, HID], f32)
                nc.vector.tensor_mul(prod[:], h1s[:], w2r_sb[:])
                red = small.tile([C, 1], f32)
                nc.vector.reduce_sum(red[:], prod[:], axis=mybir.AxisListType.X)
                cf = small.tile([C, 1], f32)
                nc.vector.tensor_add(cf[:], red[:], b2r_sb[:])
                # coef padded to 128 partitions for a K=128 matmul (small-K
                # matmuls are flaky on HW).
                coef128 = small.tile([128, 1], f32)
                nc.vector.memset(coef128[:], 0.0)
                nc.vector.scalar_tensor_tensor(
                    out=coef128[0:C, :], in0=cf[:], scalar=NEG, in1=cf[:],
                    op0=Alu.mult, op1=Alu.max,
                )
                with tc.tile_pool(name="psum_u", bufs=1, space="PSUM") as pup:
                    u_ps = pup.tile([128, 1], f32)
                    # u128[p] = coef[p%32] * d[p%32] / 32  (t128 folds d/32)
                    nc.tensor.matmul(
                        u_ps[:], lhsT=t128_sb[:], rhs=coef128[:],
                        start=True, stop=True,
                    )
                    nc.vector.tensor_scalar_mul(S_sb[:], m0_sb[:], u_ps[:])
                small_cm.__exit__(None, None, None)

            # ---- weighted channel sum ----
            # Four groups per PSUM bank via col-tiling: group (t, j) of block
            # blk lands at psum partitions [32j, 32j+4) (tile_position is
            # inferred from the out AP's base partition).  One full-bank
            # [128, 512] copy then moves 4 groups at once (vs 4-partition
            # copies), and the bf16 staging halves the SBUF-port-0/1-bound
            # read bytes of the output DMA (gpsimd cast-DMA widens to fp32).
            with (
                tc.tile_pool(name="psum_o", bufs=3, space="PSUM") as pop,
                tc.tile_pool(name="stg", bufs=2) as stgp,
            ):
                for blk in range(G // OUTB):
                    stg = stgp.tile([128, OUTB // P, W], bf16)
                    for t in range(OUTB // P):
                        po = pop.tile([128, W], f32)
                        for j in range(P):
                            g = OUTB * blk + P * t + j
                            nc.tensor.matmul(
                                po[32 * j : 32 * j + P, :], lhsT=S_sb[:],
                                rhs=xs_sb[:, W * g : W * (g + 1)],
                                start=True, stop=True,
                                tile_position=(0, 32 * j),
                            )
                        if t % 2 == 0:
                            nc.scalar.copy(stg[:, t, :], po[:])
                        else:
                            nc.vector.tensor_copy(stg[:, t, :], po[:])
                    # out row 4g+w = 64*blk + 16t + 4j + w: one DMA per j
                    # (partitions [32j, 32j+4)), DRAM side viewed as (w,t,h)
                    blk_rows = out_t[
                        P * OUTB * blk : P * OUTB * (blk + 1), :
                    ].rearrange("(t j w) h -> j w t h", j=P, w=P)
                    for j in range(P):
                        nc.gpsimd.dma_start(
                            blk_rows[j], stg[32 * j : 32 * j + P, :, :]
                        )

    nc.compile()
    return nc


def _get_program():
    if "nc" not in _prog_cache:
        _prog_cache["nc"] = _build_program()
    return _prog_cache["nc"]


def prep_in_maps(x, dist, W1, b1, W2, b2):
    bf16 = ml_dtypes.bfloat16
    x = np.asarray(x, dtype=np.float32)
    dist = np.asarray(dist, dtype=np.float32)
    W1 = np.asarray(W1, dtype=np.float32)
    b1 = np.asarray(b1, dtype=np.float32)
    W2 = np.asarray(W2, dtype=np.float32)
    b2 = np.asarray(b2, dtype=np.float32)

    d = dist / dist.sum()
    # A[(w*32+c), n] = d[c]/16 * delta(c, n): folds w-sum + avgpool norm + d
    amat = np.tile(np.diag(d / (P * P)).astype(np.float32), (P, 1))
    # M0[(w*32+c), m] = delta(w, m)
    m0 = np.repeat(np.eye(P, dtype=np.float32), C, axis=0).astype(bf16)
    # T128[c, p] = d[c]/32 * delta(c, p % 32), zero-padded to K=128 rows
    t128 = np.zeros((128, 128), np.float32)
    t128[0:C] = np.tile(np.diag(d / C).astype(np.float32), (1, P))
    b1r = np.ascontiguousarray(np.broadcast_to(b1, (C, HID))).astype(np.float32)
    w2r = np.ascontiguousarray(np.broadcast_to(W2[0], (C, HID))).astype(np.float32)
    b2r = np.full((C, 1), b2[0], dtype=np.float32)

    # w1x[j, g, hid] = W1[hid, 128g+j]  -> [128, G*HID] bf16, so each
    # partition j reads contiguous runs (one 2 KiB run per group)
    w1x = np.ascontiguousarray(
        W1.reshape(HID, G, 128).transpose(2, 1, 0)
    ).astype(bf16).reshape(128, G * HID)

    # x2[(w,c), (g,k,h)] = x[c,k,4g+w,h] -> [128, G*K*W] bf16: partition-
    # major contiguous so DMAs use 8 KiB descriptors on all 16 engines;
    # bf16 halves the dominant HBM stream (same trick as W1)
    x2 = [
        np.ascontiguousarray(
            x[b].astype(bf16).reshape(C, K, G, P, W).transpose(3, 0, 2, 1, 4)
        ).reshape(128, G * K * W)
        for b in range(N_CORES)
    ]

    return [
        dict(
            x2=x2[b],
            w1x=w1x,
            amat=amat,
            m0=m0,
            t128=t128,
            b1r=b1r,
            w2r=w2r,
            b2r=b2r,
        )
        for b in range(N_CORES)
    ]


def kernel(x, dist, W1, b1, W2, b2):
    from concourse.bass_utils import run_bass_kernel_spmd

    in_maps = prep_in_maps(x, dist, W1, b1, W2, b2)
    nc = _get_program()
    res = run_bass_kernel_spmd(nc, in_maps, list(range(N_CORES)))
    out = np.stack([res.results[i]["out"] for i in range(N_CORES)])
    return out[:, None, :, :].astype(np.float32)


# revision 22
# speedup vs baseline: 4.3091x; 1.1558x over previous
"""Trainium2 Bass kernel for nn_MixChan (dense_mlp).

Reference computation (per batch sample b):
    d   = dist / dist.sum()                       # (32,)
    xs  = x.sum(axis=K) * d[c]                    # (32, 512, 512)
    ds  = avgpool4x4(xs)                          # (32, 128, 128)
    h1  = leaky_relu(ds.flat @ W1.T + b1, 0.2)    # (32, 1024)
    coef= leaky_relu(h1 @ W2.T + b2, 0.2)         # (32, 1)
    out = einsum('c,cwh->wh', coef, xs) / 32      # (512, 512)

Sharding: data-parallel over batch B=8 across the 8 NeuronCores; the MLP
weights are replicated (W1 is streamed from HBM as bf16).

Both x and W1 are pre-transposed on the host into partition-major layouts
so every DMA is a [128-partition, big-contiguous-run] transfer: the HWDGE
splits a DMA across SDMA engines by the outer DRAM dim, so a 128-outer AP
uses all 16 engines (the old (w4,c32,k,h) transpose AP used only 4 and the
kernel ran at ~100 GB/s, DMA-bound at 930 us).

Per-core dataflow (one pass over x, fused), TILE=4 row-groups per step:
  for each tile t (32 tiles):
    - DMA x2[:, t] -> xt [128p=(w4,c32), (g4,k2,h512)] fp32 (2 MiB, 16 KiB/desc)
    - DMA w1x[:, t] -> w14 [128p=j, (g4,hid1024)] bf16 (1 MiB, 8 KiB/desc)
    - DVE: k-sum -> xs slab [128, (g,h)] bf16 (kept in SBUF, 16 MiB)
    - DVE: reduce over h-groups of 4 -> red16 [128, 512] fp32
    - PE x4: red16 chunk.T @ A -> pooled [128f, 32c] into one PSUM bank (A
      folds the w-sum, /16 avgpool norm, and the per-channel dist weight)
    - ACT: cast the 4-group pooled bank -> bf16
    - PE x8: h1 += pooled.T @ W1 chunks (PSUM accumulate over all 128 chunks)
  MLP tail (tiny), building S [128, 4] = u_c * delta(w, m), u = coef*d/32
  for each group g: (output phase)
    - PE : out rows = S.T @ xs slab slice  (the weighted channel sum)
    - ACT/DVE alternating: copy PSUM -> staging [4, 16*512]
    - every 16 groups: one batched DMA staging -> out rows (128 KiB)
"""

import numpy as np
import ml_dtypes

B, C, K, W = 8, 32, 2, 512
P = 4                 # pool kernel/stride
G = W // P            # 128 row groups
FEAT = G * G          # 16384
HID = 1024
NEG = 0.2
N_CORES = 8
TILE = 4              # row groups per main-loop tile
NT = G // TILE        # 32 tiles
OUTB = 16             # groups per output staging buffer

_prog_cache = {}


def _build_program():
    import concourse.bass as bass
    import concourse.tile as tile
    from concourse import bacc, mybir

    dt = mybir.dt
    f32 = dt.float32
    bf16 = dt.bfloat16
    Alu = mybir.AluOpType

    nc = bacc.Bacc(
        "TRN2",
        debug=False,
        enable_asserts=False,
        target_bir_lowering=False,
        num_devices=N_CORES,
    )

    # x2[(w*32+c), ((g*2+k)*512+h)] = x[c,k,4g+w,h]   (host pre-transposed,
    # cast to bf16 host-side — same traffic-halving trick as W1)
    x2_t = nc.dram_tensor("x2", [128, G * K * W], bf16, kind="ExternalInput").ap()
    # w1x[j, (g*1024+hid)] = W1[hid, 128g+j]          (host pre-transposed)
    w1x_t = nc.dram_tensor("w1x", [128, G * HID], bf16, kind="ExternalInput").ap()
    amat_t = nc.dram_tensor("amat", [128, C], bf16, kind="ExternalInput").ap()
    m0_t = nc.dram_tensor("m0", [128, P], bf16, kind="ExternalInput").ap()
    t128_t = nc.dram_tensor("t128", [128, 128], f32, kind="ExternalInput").ap()
    b1r_t = nc.dram_tensor("b1r", [C, HID], f32, kind="ExternalInput").ap()
    w2r_t = nc.dram_tensor("w2r", [C, HID], f32, kind="ExternalInput").ap()
    b2r_t = nc.dram_tensor("b2r", [C, 1], f32, kind="ExternalInput").ap()
    out_t = nc.dram_tensor("out", [W, W], f32, kind="ExternalOutput").ap()

    with tile.TileContext(nc) as tc:
        with tc.tile_pool(name="singles", bufs=1) as singles:
            # constants — loaded via the gpsimd (SWDGE) queue so the sync /
            # scalar HWDGE rings start streaming x / W1 from cycle 0
            amat_sb = singles.tile([128, C], bf16)
            nc.gpsimd.dma_start(amat_sb[:], amat_t)
            m0_sb = singles.tile([128, P], bf16)
            nc.gpsimd.dma_start(m0_sb[:], m0_t)
            t128_sb = singles.tile([128, 128], f32)
            nc.gpsimd.dma_start(t128_sb[:], t128_t)
            b1r_sb = singles.tile([C, HID], f32)
            nc.gpsimd.dma_start(b1r_sb[:], b1r_t)
            w2r_sb = singles.tile([C, HID], f32)
            nc.gpsimd.dma_start(w2r_sb[:], w2r_t)
            b2r_sb = singles.tile([C, 1], f32)
            nc.gpsimd.dma_start(b2r_sb[:], b2r_t)

            # xs slab: k-summed (unscaled) x, bf16, [(w4 c32), (g128 h512)]
            xs_sb = singles.tile([128, G * W], bf16)
            S_sb = singles.tile([128, P], bf16)

            with tc.tile_pool(name="psum_h1", bufs=1, space="PSUM") as ph1:
                h1p = ph1.tile([C, HID], f32)

                with (
                    tc.tile_pool(name="xt", bufs=3) as xtp,
                    tc.tile_pool(name="w1c", bufs=3) as w1p,
                    tc.tile_pool(name="red", bufs=2) as redp,
                    tc.tile_pool(name="ptsb", bufs=2) as ptp,
                    tc.tile_pool(name="psum_pt", bufs=2, space="PSUM") as pptp,
                ):
                    for t in range(NT):
                        xt = xtp.tile([128, TILE, K, W], bf16)
                        nc.sync.dma_start(
                            xt[:],
                            x2_t[:, TILE * K * W * t : TILE * K * W * (t + 1)],
                        )
                        w14 = w1p.tile([128, TILE, HID], bf16)
                        nc.scalar.dma_start(
                            w14[:],
                            w1x_t[:, TILE * HID * t : TILE * HID * (t + 1)],
                        )

                        # k-sum -> xs (bf16), one instr for the whole tile
                        xs_slice = xs_sb[
                            :, TILE * W * t : TILE * W * (t + 1)
                        ].rearrange("p (g h) -> p g h", h=W)
                        nc.vector.tensor_add(
                            xs_slice, xt[:, :, 0, :], xt[:, :, 1, :]
                        )

                        # 4:1 reduce over h within pooling groups (bf16 so
                        # the pooled matmul avoids the fp32 LOW_HIGH 2-pass)
                        red16 = redp.tile([128, TILE * G], bf16)
                        with nc.allow_low_precision(
                            reason="4-term bf16 pool sum, fp32 internal"
                        ):
                            nc.vector.reduce_sum(
                                red16[:],
                                xs_sb[
                                    :, TILE * W * t : TILE * W * (t + 1)
                                ].rearrange("p (j f) -> p j f", f=P),
                                axis=mybir.AxisListType.X,
                            )

                        # pooled features for the 4 groups -> one PSUM bank
                        pp4 = pptp.tile([128, TILE, C], f32)
                        for q in range(TILE):
                            nc.tensor.matmul(
                                pp4[:, q, :],
                                lhsT=red16[:, G * q : G * (q + 1)],
                                rhs=amat_sb[:],
                                start=True,
                                stop=True,
                            )
                        pt4 = ptp.tile([128, TILE, C], bf16)
                        nc.scalar.copy(pt4[:], pp4[:])

                        for q in range(TILE):
                            first = t == 0 and q == 0
                            last = t == NT - 1 and q == TILE - 1
                            nc.tensor.matmul(
                                h1p[:, 0:512],
                                lhsT=pt4[:, q, :],
                                rhs=w14[:, q, 0:512],
                                start=first,
                                stop=last,
                            )
                            nc.tensor.matmul(
                                h1p[:, 512:1024],
                                lhsT=pt4[:, q, :],
                                rhs=w14[:, q, 512:1024],
                                start=first,
                                stop=last,
                            )

                # ---- MLP tail ----
                small_cm = tc.tile_pool(name="small", bufs=1)
                small = small_cm.__enter__()
                tmp1 = small.tile([C, HID], f32)
                nc.vector.tensor_add(tmp1[:], h1p[:], b1r_sb[:])
                h1s = small.tile([C, HID], f32)
                nc.vector.scalar_tensor_tensor(
                    out=h1s[:], in0=tmp1[:], scalar=NEG, in1=tmp1[:],
                    op0=Alu.mult, op1=Alu.max,
                )
                # coef = leaky(h1s @ W2.T + b2): tensor_tensor_reduce crashes
                # the exec unit on HW, so mul + reduce + add instead.
                prod = small.tile([C, HID], f32)
                nc.vector.tensor_mul(prod[:], h1s[:], w2r_sb[:])
                red = small.tile([C, 1], f32)
                nc.vector.reduce_sum(red[:], prod[:], axis=mybir.AxisListType.X)
                cf = small.tile([C, 1], f32)
                nc.vector.tensor_add(cf[:], red[:], b2r_sb[:])
                # coef padded to 128 partitions for a K=128 matmul (small-K
                # matmuls are flaky on HW).
                coef128 = small.tile([128, 1], f32)
                nc.vector.memset(coef128[:], 0.0)
                nc.vector.scalar_tensor_tensor(
                    out=coef128[0:C, :], in0=cf[:], scalar=NEG, in1=cf[:],
                    op0=Alu.mult, op1=Alu.max,
                )
                with tc.tile_pool(name="psum_u", bufs=1, space="PSUM") as pup:
                    u_ps = pup.tile([128, 1], f32)
                    # u128[p] = coef[p%32] * d[p%32] / 32  (t128 folds d/32)
                    nc.tensor.matmul(
                        u_ps[:], lhsT=t128_sb[:], rhs=coef128[:],
                        start=True, stop=True,
                    )
                    nc.vector.tensor_scalar_mul(S_sb[:], m0_sb[:], u_ps[:])
                small_cm.__exit__(None, None, None)

            # ---- weighted channel sum ----
            # Four groups per PSUM bank via col-tiling: group (t, j) of block
            # blk lands at psum partitions [32j, 32j+4) (tile_position is
            # inferred from the out AP's base partition).  One full-bank
            # [128, 512] copy then moves 4 groups at once (vs 4-partition
            # copies), and the bf16 staging halves the SBUF-port-0/1-bound
            # read bytes of the output DMA (gpsimd cast-DMA widens to fp32).
            with (
                tc.tile_pool(name="psum_o", bufs=4, space="PSUM") as pop,
                tc.tile_pool(name="stg", bufs=2) as stgp,
            ):
                for blk in range(G // OUTB):
                    stg = stgp.tile([128, OUTB // P, W], bf16)
                    for t in range(OUTB // P):
                        po = pop.tile([128, W], f32)
                        for j in range(P):
                            g = OUTB * blk + P * t + j
                            nc.tensor.matmul(
                                po[32 * j : 32 * j + P, :], lhsT=S_sb[:],
                                rhs=xs_sb[:, W * g : W * (g + 1)],
                                start=True, stop=True,
                                tile_position=(0, 32 * j),
                            )
                        if t % 2 == 0:
                            nc.scalar.copy(stg[:, t, :], po[:])
                        else:
                            nc.vector.tensor_copy(stg[:, t, :], po[:])
                    # out row 4g+w = 64*blk + 16t + 4j + w: one DMA per j
                    # (partitions [32j, 32j+4)), DRAM side viewed as (w,t,h)
                    blk_rows = out_t[
                        P * OUTB * blk : P * OUTB * (blk + 1), :
                    ].rearrange("(t j w) h -> j w t h", j=P, w=P)
                    for j in range(P):
                        nc.gpsimd.dma_start(
                            blk_rows[j], stg[32 * j : 32 * j + P, :, :]
                        )

    nc.compile()
    return nc


def _get_program():
    if "nc" not in _prog_cache:
        _prog_cache["nc"] = _build_program()
    return _prog_cache["nc"]


def prep_in_maps(x, dist, W1, b1, W2, b2):
    bf16 = ml_dtypes.bfloat16
    x = np.asarray(x, dtype=np.float32)
    dist = np.asarray(dist, dtype=np.float32)
    W1 = np.asarray(W1, dtype=np.float32)
    b1 = np.asarray(b1, dtype=np.float32)
    W2 = np.asarray(W2, dtype=np.float32)
    b2 = np.asarray(b2, dtype=np.float32)

    d = dist / dist.sum()
    # A[(w*32+c), n] = d[c]/16 * delta(c, n): folds w-sum + avgpool norm + d
    amat = np.tile(np.diag(d / (P * P)).astype(np.float32), (P, 1)).astype(bf16)
    # M0[(w*32+c), m] = delta(w, m)
    m0 = np.repeat(np.eye(P, dtype=np.float32), C, axis=0).astype(bf16)
    # T128[c, p] = d[c]/32 * delta(c, p % 32), zero-padded to K=128 rows
    t128 = np.zeros((128, 128), np.float32)
    t128[0:C] = np.tile(np.diag(d / C).astype(np.float32), (1, P))
    b1r = np.ascontiguousarray(np.broadcast_to(b1, (C, HID))).astype(np.float32)
    w2r = np.ascontiguousarray(np.broadcast_to(W2[0], (C, HID))).astype(np.float32)
    b2r = np.full((C, 1), b2[0], dtype=np.float32)

    # w1x[j, g, hid] = W1[hid, 128g+j]  -> [128, G*HID] bf16, so each
    # partition j reads contiguous runs (one 2 KiB run per group)
    w1x = np.ascontiguousarray(
        W1.reshape(HID, G, 128).transpose(2, 1, 0)
    ).astype(bf16).reshape(128, G * HID)

    # x2[(w,c), (g,k,h)] = x[c,k,4g+w,h] -> [128, G*K*W] bf16: partition-
    # major contiguous so DMAs use 8 KiB descriptors on all 16 engines;
    # bf16 halves the dominant HBM stream (same trick as W1)
    x2 = [
        np.ascontiguousarray(
            x[b].astype(bf16).reshape(C, K, G, P, W).transpose(3, 0, 2, 1, 4)
        ).reshape(128, G * K * W)
        for b in range(N_CORES)
    ]

    return [
        dict(
            x2=x2[b],
            w1x=w1x,
            amat=amat,
            m0=m0,
            t128=t128,
            b1r=b1r,
            w2r=w2r,
            b2r=b2r,
        )
        for b in range(N_CORES)
    ]


def kernel(x, dist, W1, b1, W2, b2):
    from concourse.bass_utils import run_bass_kernel_spmd

    in_maps = prep_in_maps(x, dist, W1, b1, W2, b2)
    nc = _get_program()
    res = run_bass_kernel_spmd(nc, in_maps, list(range(N_CORES)))
    out = np.stack([res.results[i]["out"] for i in range(N_CORES)])
    return out[:, None, :, :].astype(np.float32)


# revision 25
# speedup vs baseline: 4.3800x; 1.0165x over previous
"""Trainium2 Bass kernel for nn_MixChan (dense_mlp).

Reference computation (per batch sample b):
    d   = dist / dist.sum()                       # (32,)
    xs  = x.sum(axis=K) * d[c]                    # (32, 512, 512)
    ds  = avgpool4x4(xs)                          # (32, 128, 128)
    h1  = leaky_relu(ds.flat @ W1.T + b1, 0.2)    # (32, 1024)
    coef= leaky_relu(h1 @ W2.T + b2, 0.2)         # (32, 1)
    out = einsum('c,cwh->wh', coef, xs) / 32      # (512, 512)

Sharding: data-parallel over batch B=8 across the 8 NeuronCores; the MLP
weights are replicated (W1 is streamed from HBM as bf16).

Both x and W1 are pre-transposed on the host into partition-major layouts
so every DMA is a [128-partition, big-contiguous-run] transfer: the HWDGE
splits a DMA across SDMA engines by the outer DRAM dim, so a 128-outer AP
uses all 16 engines (the old (w4,c32,k,h) transpose AP used only 4 and the
kernel ran at ~100 GB/s, DMA-bound at 930 us).

Per-core dataflow (one pass over x, fused), TILE=4 row-groups per step:
  for each tile t (32 tiles):
    - DMA x2[:, t] -> xt [128p=(w4,c32), (g4,k2,h512)] fp32 (2 MiB, 16 KiB/desc)
    - DMA w1x[:, t] -> w14 [128p=j, (g4,hid1024)] bf16 (1 MiB, 8 KiB/desc)
    - DVE: k-sum -> xs slab [128, (g,h)] bf16 (kept in SBUF, 16 MiB)
    - DVE: reduce over h-groups of 4 -> red16 [128, 512] fp32
    - PE x4: red16 chunk.T @ A -> pooled [128f, 32c] into one PSUM bank (A
      folds the w-sum, /16 avgpool norm, and the per-channel dist weight)
    - ACT: cast the 4-group pooled bank -> bf16
    - PE x8: h1 += pooled.T @ W1 chunks (PSUM accumulate over all 128 chunks)
  MLP tail (tiny), building S [128, 4] = u_c * delta(w, m), u = coef*d/32
  for each group g: (output phase)
    - PE : out rows = S.T @ xs slab slice  (the weighted channel sum)
    - ACT/DVE alternating: copy PSUM -> staging [4, 16*512]
    - every 16 groups: one batched DMA staging -> out rows (128 KiB)
"""

import numpy as np
import ml_dtypes

B, C, K, W = 8, 32, 2, 512
P = 4                 # pool kernel/stride
G = W // P            # 128 row groups
FEAT = G * G          # 16384
HID = 1024
NEG = 0.2
N_CORES = 8
TILE = 4              # row groups per main-loop tile
NT = G // TILE        # 32 tiles
OUTB = 16             # groups per output staging buffer

_prog_cache = {}


def _build_program():
    import concourse.bass as bass
    import concourse.tile as tile
    from concourse import bacc, mybir

    dt = mybir.dt
    f32 = dt.float32
    bf16 = dt.bfloat16
    Alu = mybir.AluOpType

    nc = bacc.Bacc(
        "TRN2",
        debug=False,
        enable_asserts=False,
        target_bir_lowering=False,
        num_devices=N_CORES,
    )

    # x2[(w*32+c), ((g*2+k)*512+h)] = x[c,k,4g+w,h]   (host pre-transposed,
    # cast to bf16 host-side — same traffic-halving trick as W1)
    x2_t = nc.dram_tensor("x2", [128, G * K * W], bf16, kind="ExternalInput").ap()
    # w1x[j, (g*1024+hid)] = W1[hid, 128g+j]          (host pre-transposed)
    w1x_t = nc.dram_tensor("w1x", [128, G * HID], bf16, kind="ExternalInput").ap()
    amat_t = nc.dram_tensor("amat", [128, C], bf16, kind="ExternalInput").ap()
    m0_t = nc.dram_tensor("m0", [128, P], bf16, kind="ExternalInput").ap()
    t128_t = nc.dram_tensor("t128", [128, 128], f32, kind="ExternalInput").ap()
    b1r_t = nc.dram_tensor("b1r", [C, HID], f32, kind="ExternalInput").ap()
    w2r_t = nc.dram_tensor("w2r", [C, HID], f32, kind="ExternalInput").ap()
    b2r_t = nc.dram_tensor("b2r", [C, 1], f32, kind="ExternalInput").ap()
    out_t = nc.dram_tensor("out", [W, W], f32, kind="ExternalOutput").ap()

    with tile.TileContext(nc) as tc:
        with tc.tile_pool(name="singles", bufs=1) as singles:
            # constants — loaded via the gpsimd (SWDGE) queue so the sync /
            # scalar HWDGE rings start streaming x / W1 from cycle 0
            amat_sb = singles.tile([128, C], bf16)
            nc.gpsimd.dma_start(amat_sb[:], amat_t)
            m0_sb = singles.tile([128, P], bf16)
            nc.gpsimd.dma_start(m0_sb[:], m0_t)
            t128_sb = singles.tile([128, 128], f32)
            nc.gpsimd.dma_start(t128_sb[:], t128_t)
            b1r_sb = singles.tile([C, HID], f32)
            nc.gpsimd.dma_start(b1r_sb[:], b1r_t)
            w2r_sb = singles.tile([C, HID], f32)
            nc.gpsimd.dma_start(w2r_sb[:], w2r_t)
            b2r_sb = singles.tile([C, 1], f32)
            nc.gpsimd.dma_start(b2r_sb[:], b2r_t)

            # xs slab: k-summed (unscaled) x, bf16, [(w4 c32), (g128 h512)]
            xs_sb = singles.tile([128, G * W], bf16)
            S_sb = singles.tile([128, P], bf16)

            with tc.tile_pool(name="psum_h1", bufs=1, space="PSUM") as ph1:
                h1p = ph1.tile([C, HID], f32)

                with (
                    tc.tile_pool(name="xt", bufs=4) as xtp,
                    tc.tile_pool(name="w1c", bufs=3) as w1p,
                    tc.tile_pool(name="red", bufs=2) as redp,
                    tc.tile_pool(name="ptsb", bufs=2) as ptp,
                    tc.tile_pool(name="psum_pt", bufs=2, space="PSUM") as pptp,
                ):
                    for t in range(NT):
                        xt = xtp.tile([128, TILE, K, W], bf16)
                        nc.sync.dma_start(
                            xt[:],
                            x2_t[:, TILE * K * W * t : TILE * K * W * (t + 1)],
                        )
                        w14 = w1p.tile([128, TILE, HID], bf16)
                        nc.scalar.dma_start(
                            w14[:],
                            w1x_t[:, TILE * HID * t : TILE * HID * (t + 1)],
                        )

                        # k-sum -> xs (bf16), one instr for the whole tile
                        xs_slice = xs_sb[
                            :, TILE * W * t : TILE * W * (t + 1)
                        ].rearrange("p (g h) -> p g h", h=W)
                        nc.vector.tensor_add(
                            xs_slice, xt[:, :, 0, :], xt[:, :, 1, :]
                        )

                        # 4:1 reduce over h within pooling groups (bf16 so
                        # the pooled matmul avoids the fp32 LOW_HIGH 2-pass)
                        red16 = redp.tile([128, TILE * G], bf16)
                        with nc.allow_low_precision(
                            reason="4-term bf16 pool sum, fp32 internal"
                        ):
                            nc.vector.reduce_sum(
                                red16[:],
                                xs_sb[
                                    :, TILE * W * t : TILE * W * (t + 1)
                                ].rearrange("p (j f) -> p j f", f=P),
                                axis=mybir.AxisListType.X,
                            )

                        # pooled features for the 4 groups -> one PSUM bank
                        pp4 = pptp.tile([128, TILE, C], f32)
                        for q in range(TILE):
                            nc.tensor.matmul(
                                pp4[:, q, :],
                                lhsT=red16[:, G * q : G * (q + 1)],
                                rhs=amat_sb[:],
                                start=True,
                                stop=True,
                            )
                        pt4 = ptp.tile([128, TILE, C], bf16)
                        nc.scalar.copy(pt4[:], pp4[:])

                        for q in range(TILE):
                            first = t == 0 and q == 0
                            last = t == NT - 1 and q == TILE - 1
                            nc.tensor.matmul(
                                h1p[:, 0:512],
                                lhsT=pt4[:, q, :],
                                rhs=w14[:, q, 0:512],
                                start=first,
                                stop=last,
                            )
                            nc.tensor.matmul(
                                h1p[:, 512:1024],
                                lhsT=pt4[:, q, :],
                                rhs=w14[:, q, 512:1024],
                                start=first,
                                stop=last,
                            )

                # PE warm-up: fat (M=128) matmuls over already-resident xs
                # slices, overlapping the DVE-only MLP tail, so the HAM
                # clock gate is at K=8/8 when the output matmuls start.
                # They depend on late xs slices so they cannot run early.
                with tc.tile_pool(name="psum_w", bufs=1, space="PSUM") as pwp:
                    warm = pwp.tile([128, W], f32)
                    for i in range(6):
                        nc.tensor.matmul(
                            warm[:],
                            lhsT=xs_sb[:, W * (G - 2) : W * (G - 2) + 128],
                            rhs=xs_sb[:, W * (G - 1) : W * G],
                            start=True, stop=True,
                        )

                # ---- MLP tail ----
                small_cm = tc.tile_pool(name="small", bufs=1)
                small = small_cm.__enter__()
                tmp1 = small.tile([C, HID], f32)
                nc.vector.tensor_add(tmp1[:], h1p[:], b1r_sb[:])
                h1s = small.tile([C, HID], f32)
                nc.vector.scalar_tensor_tensor(
                    out=h1s[:], in0=tmp1[:], scalar=NEG, in1=tmp1[:],
                    op0=Alu.mult, op1=Alu.max,
                )
                # coef = leaky(h1s @ W2.T + b2): tensor_tensor_reduce crashes
                # the exec unit on HW, so mul + reduce + add instead.
                prod = small.tile([C, HID], f32)
                nc.vector.tensor_mul(prod[:], h1s[:], w2r_sb[:])
                red = small.tile([C, 1], f32)
                nc.vector.reduce_sum(red[:], prod[:], axis=mybir.AxisListType.X)
                cf = small.tile([C, 1], f32)
                nc.vector.tensor_add(cf[:], red[:], b2r_sb[:])
                # coef padded to 128 partitions for a K=128 matmul (small-K
                # matmuls are flaky on HW).
                coef128 = small.tile([128, 1], f32)
                nc.vector.memset(coef128[:], 0.0)
                nc.vector.scalar_tensor_tensor(
                    out=coef128[0:C, :], in0=cf[:], scalar=NEG, in1=cf[:],
                    op0=Alu.mult, op1=Alu.max,
                )
                with tc.tile_pool(name="psum_u", bufs=1, space="PSUM") as pup:
                    u_ps = pup.tile([128, 1], f32)
                    # u128[p] = coef[p%32] * d[p%32] / 32  (t128 folds d/32)
                    nc.tensor.matmul(
                        u_ps[:], lhsT=t128_sb[:], rhs=coef128[:],
                        start=True, stop=True,
                    )
                    nc.vector.tensor_scalar_mul(S_sb[:], m0_sb[:], u_ps[:])
                small_cm.__exit__(None, None, None)

            # ---- weighted channel sum ----
            # Four groups per PSUM bank via col-tiling: group (t, j) of block
            # blk lands at psum partitions [32j, 32j+4) (tile_position is
            # inferred from the out AP's base partition).  One full-bank
            # [128, 512] copy then moves 4 groups at once (vs 4-partition
            # copies), and the bf16 staging halves the SBUF-port-0/1-bound
            # read bytes of the output DMA (gpsimd cast-DMA widens to fp32).
            with (
                tc.tile_pool(name="psum_o", bufs=4, space="PSUM") as pop,
                tc.tile_pool(name="stg", bufs=3) as stgp,
            ):
                for blk in range(G // OUTB):
                    stg = stgp.tile([128, OUTB // P, W], bf16)
                    for t in range(OUTB // P):
                        po = pop.tile([128, W], f32)
                        for j in range(P):
                            g = OUTB * blk + P * t + j
                            nc.tensor.matmul(
                                po[32 * j : 32 * j + P, :], lhsT=S_sb[:],
                                rhs=xs_sb[:, W * g : W * (g + 1)],
                                start=True, stop=True,
                                tile_position=(0, 32 * j),
                            )
                        if t % 2 == 0:
                            nc.scalar.copy(stg[:, t, :], po[:])
                        else:
                            nc.vector.tensor_copy(stg[:, t, :], po[:])
                    # out row 4g+w = 64*blk + 16t + 4j + w: one DMA per j
                    # (partitions [32j, 32j+4)), DRAM side viewed as (w,t,h)
                    blk_rows = out_t[
                        P * OUTB * blk : P * OUTB * (blk + 1), :
                    ].rearrange("(t j w) h -> j w t h", j=P, w=P)
                    for j in range(P):
                        nc.gpsimd.dma_start(
                            blk_rows[j], stg[32 * j : 32 * j + P, :, :]
                        )

    nc.compile()
    return nc


def _get_program():
    if "nc" not in _prog_cache:
        _prog_cache["nc"] = _build_program()
    return _prog_cache["nc"]


def prep_in_maps(x, dist, W1, b1, W2, b2):
    bf16 = ml_dtypes.bfloat16
    x = np.asarray(x, dtype=np.float32)
    dist = np.asarray(dist, dtype=np.float32)
    W1 = np.asarray(W1, dtype=np.float32)
    b1 = np.asarray(b1, dtype=np.float32)
    W2 = np.asarray(W2, dtype=np.float32)
    b2 = np.asarray(b2, dtype=np.float32)

    d = dist / dist.sum()
    # A[(w*32+c), n] = d[c]/16 * delta(c, n): folds w-sum + avgpool norm + d
    amat = np.tile(np.diag(d / (P * P)).astype(np.float32), (P, 1)).astype(bf16)
    # M0[(w*32+c), m] = delta(w, m)
    m0 = np.repeat(np.eye(P, dtype=np.float32), C, axis=0).astype(bf16)
    # T128[c, p] = d[c]/32 * delta(c, p % 32), zero-padded to K=128 rows
    t128 = np.zeros((128, 128), np.float32)
    t128[0:C] = np.tile(np.diag(d / C).astype(np.float32), (1, P))
    b1r = np.ascontiguousarray(np.broadcast_to(b1, (C, HID))).astype(np.float32)
    w2r = np.ascontiguousarray(np.broadcast_to(W2[0], (C, HID))).astype(np.float32)
    b2r = np.full((C, 1), b2[0], dtype=np.float32)

    # w1x[j, g, hid] = W1[hid, 128g+j]  -> [128, G*HID] bf16, so each
    # partition j reads contiguous runs (one 2 KiB run per group)
    w1x = np.ascontiguousarray(
        W1.reshape(HID, G, 128).transpose(2, 1, 0)
    ).astype(bf16).reshape(128, G * HID)

    # x2[(w,c), (g,k,h)] = x[c,k,4g+w,h] -> [128, G*K*W] bf16: partition-
    # major contiguous so DMAs use 8 KiB descriptors on all 16 engines;
    # bf16 halves the dominant HBM stream (same trick as W1)
    x2 = [
        np.ascontiguousarray(
            x[b].astype(bf16).reshape(C, K, G, P, W).transpose(3, 0, 2, 1, 4)
        ).reshape(128, G * K * W)
        for b in range(N_CORES)
    ]

    return [
        dict(
            x2=x2[b],
            w1x=w1x,
            amat=amat,
            m0=m0,
            t128=t128,
            b1r=b1r,
            w2r=w2r,
            b2r=b2r,
        )
        for b in range(N_CORES)
    ]


def kernel(x, dist, W1, b1, W2, b2):
    from concourse.bass_utils import run_bass_kernel_spmd

    in_maps = prep_in_maps(x, dist, W1, b1, W2, b2)
    nc = _get_program()
    res = run_bass_kernel_spmd(nc, in_maps, list(range(N_CORES)))
    out = np.stack([res.results[i]["out"] for i in range(N_CORES)])
    return out[:, None, :, :].astype(np.float32)
